# revision 1
# baseline (speedup 1.0000x reference)
"""Trainium2 Bass kernel for nn_Decoder_25718264168590.

2-layer LSTM decoder (B=32, T=50, H=1024, E=128) + vocab projection
(V=32000) + log_softmax, distributed over 8 NeuronCores:

- LSTM: gate-sharded 8 ways (core r owns h-units [r*128, (r+1)*128) of
  both layers = 512 gate rows each).  Activations live in
  [feature-on-partition, (ktile, batch)] layout so the recurrent matmuls
  use stationary weight tiles and no transposes anywhere.  The input
  projection (concat([x_t, enc]) @ proj_w.T) @ w_ih0.T is algebraically
  folded: the x part becomes a 9th K-tile of the layer-0 matmul
  (A1 = P1 @ w_ih0.T), the per-sample encoder part plus biases enter the
  PSUM accumulation as a K=32 matmul against an identity matrix.
  Layer 1 runs one step behind layer 0 so each tick needs exactly ONE
  AllGather that carries [h0_t | h1_{t-1}] together, and every matmul of
  a tick depends only on the previous tick's AllGather.
- Vocab projection: lin_w sharded column-wise (4000 cols/core), resident
  in SBUF; batched matmul over all 1600 samples; log_softmax uses one
  fused exp+accumulate on ScalarE per 128-sample tile plus a small
  AllReduce per group of tiles for the cross-core sum (no max pass:
  logits are O(1) so exp cannot overflow).  Output rows are written in
  the reference's (b*T + t) order by a strided DMA.
"""

import sys

for _p in ("/opt/trn_rl_repo",):
    if _p not in sys.path:
        sys.path.insert(0, _p)

import numpy as np
import ml_dtypes

B, T, H, E, V = 32, 50, 1024, 128, 32000
NCORES = 8
VS = V // NCORES          # 4000 vocab cols per core
S = B * T                 # 1600 samples, t-major on device: s = t*32 + b
KT = H // 128             # 8 k-tiles of hidden
MT = 4                    # 4 gate m-tiles per core (i, f, o, g)
GPERM = (0, 1, 3, 2)      # torch gate order i,f,g,o -> our col order i,f,o,g
NMT = 13                  # sample m-tiles in vocab phase (12*128 + 64)
NCHK = 8                  # vocab col chunks per core (8 * 500)
CHUNK = VS // NCHK        # 500
AR_CHUNKS = ((0, 4), (4, 8), (8, 11), (11, 13))  # lse AllReduce chunking

BF16 = ml_dtypes.bfloat16

_BUILD_CACHE = {}


def _host_prep(inputs):
    """Fold projections and lay out per-core device arrays."""
    enc = np.asarray(inputs["enc_output"], np.float32)       # (B, H)
    target = np.asarray(inputs["target"], np.float32)        # (B, T, E)
    proj_w = np.asarray(inputs["proj_w"], np.float32)        # (E, H+E)
    proj_b = np.asarray(inputs["proj_b"], np.float32)        # (E,)
    w_ih0 = np.asarray(inputs["w_ih0"], np.float32)          # (4H, E)
    w_hh0 = np.asarray(inputs["w_hh0"], np.float32)          # (4H, H)
    b0 = np.asarray(inputs["b_ih0"], np.float32) + np.asarray(inputs["b_hh0"], np.float32)
    w_ih1 = np.asarray(inputs["w_ih1"], np.float32)          # (4H, H)
    w_hh1 = np.asarray(inputs["w_hh1"], np.float32)          # (4H, H)
    b1 = np.asarray(inputs["b_ih1"], np.float32) + np.asarray(inputs["b_hh1"], np.float32)
    lin_w = np.asarray(inputs["lin_w"], np.float32)          # (V, H)
    lin_b = np.asarray(inputs["lin_b"], np.float32)          # (V,)

    P1 = proj_w[:, :E].T                                     # (E, E)
    P2 = proj_w[:, E:].T                                     # (H, E)
    A1 = P1 @ w_ih0.T                                        # (E, 4H) x-path fold
    genc = (enc @ P2 + proj_b) @ w_ih0.T                     # (B, 4H) enc-path fold

    # t-major input features: xt[e, t*32+b] = target[b, t, e]
    xt = np.ascontiguousarray(
        target.transpose(1, 0, 2).reshape(S, E).T).astype(BF16)          # (128, 1600)

    # exchange-layout encoder init, [p, k*64 + c] with c<32 the h0 half and
    # c>=32 the h1 half (both init to enc): encx[p, k*64+{b,32+b}] = enc[b, k*128+p]
    e3 = enc.T.reshape(KT, 128, B).transpose(1, 0, 2)        # (128, KT, B)
    encx_bf = np.ascontiguousarray(
        np.concatenate([e3, e3], axis=2).reshape(128, KT * 2 * B)).astype(BF16)

    lin_wT = lin_w.T                                         # (H, V)
    ident = np.eye(B, dtype=BF16)

    in_maps = []
    for r in range(NCORES):
        rows = np.concatenate(
            [np.arange(128) + g * H + r * 128 for g in GPERM])           # 512 gate rows
        m = {}
        m["whh0t"] = np.ascontiguousarray(
            w_hh0[rows].T.reshape(KT, 128, 4 * 128)).astype(BF16)
        m["a1"] = np.ascontiguousarray(A1[:, rows]).astype(BF16)         # (128, 512)
        m["wih1t"] = np.ascontiguousarray(
            w_ih1[rows].T.reshape(KT, 128, 4 * 128)).astype(BF16)
        m["whh1t"] = np.ascontiguousarray(
            w_hh1[rows].T.reshape(KT, 128, 4 * 128)).astype(BF16)
        m["xt"] = xt
        # per-sample layer-0 bias (enc path + b0): gb0[b, m] feeds the PSUM
        # via a K=32 matmul against the identity.
        m["gb0"] = np.ascontiguousarray(genc[:, rows] + b0[rows]).astype(BF16)
        m["gb1"] = np.ascontiguousarray(
            np.broadcast_to(b1[rows], (B, 512))).astype(BF16)
        m["ident"] = ident
        m["encx"] = encx_bf
        m["cinit"] = np.ascontiguousarray(enc.T[r * 128:(r + 1) * 128])  # (128, 32) f32
        lw = lin_wT[:, r * VS:(r + 1) * VS]                              # (H, 4000)
        lwk = np.zeros((KT + 1, 128, VS), np.float32)
        for k in range(KT):
            lwk[k] = lw[k * 128:(k + 1) * 128]
        lwk[KT, 0, :] = lin_b[r * VS:(r + 1) * VS]
        m["linwt"] = lwk.astype(BF16)
        in_maps.append(m)
    return in_maps


def _build(reps=1):
    import concourse.bass as bass
    import concourse.tile as tile
    from concourse import bacc, mybir
    from contextlib import ExitStack

    f32 = mybir.dt.float32
    bf16 = mybir.dt.bfloat16
    AF = mybir.ActivationFunctionType
    ALU = mybir.AluOpType

    nc = bacc.Bacc("TRN2", target_bir_lowering=False, debug=False,
                   num_devices=NCORES)

    d_whh0 = nc.dram_tensor("whh0t", [KT, 128, 512], bf16, kind="ExternalInput")
    d_a1 = nc.dram_tensor("a1", [128, 512], bf16, kind="ExternalInput")
    d_wih1 = nc.dram_tensor("wih1t", [KT, 128, 512], bf16, kind="ExternalInput")
    d_whh1 = nc.dram_tensor("whh1t", [KT, 128, 512], bf16, kind="ExternalInput")
    d_xt = nc.dram_tensor("xt", [128, S], bf16, kind="ExternalInput")
    d_gb0 = nc.dram_tensor("gb0", [B, 512], bf16, kind="ExternalInput")
    d_gb1 = nc.dram_tensor("gb1", [B, 512], bf16, kind="ExternalInput")
    d_id = nc.dram_tensor("ident", [B, B], bf16, kind="ExternalInput")
    d_encx = nc.dram_tensor("encx", [128, 512], bf16, kind="ExternalInput")
    d_cinit = nc.dram_tensor("cinit", [128, 32], f32, kind="ExternalInput")
    d_linwt = nc.dram_tensor("linwt", [KT + 1, 128, VS], bf16, kind="ExternalInput")
    d_out = nc.dram_tensor("out", [S, VS], f32, kind="ExternalOutput")

    rg = [list(range(NCORES))]

    with tile.TileContext(nc) as tc, ExitStack() as ctx:
        wp = ctx.enter_context(tc.tile_pool(name="w", bufs=1))
        dp = ctx.enter_context(tc.tile_pool(name="db", bufs=6, space="DRAM"))
        hp = ctx.enter_context(tc.tile_pool(name="hx", bufs=3))
        cp = ctx.enter_context(tc.tile_pool(name="ct", bufs=2))
        tp = ctx.enter_context(tc.tile_pool(name="tmp", bufs=4))

        whh0 = [wp.tile([128, 512], bf16, name=f"whh0_{k}") for k in range(KT)]
        a1 = wp.tile([128, 512], bf16, name="a1s")
        wih1 = [wp.tile([128, 512], bf16, name=f"wih1_{k}") for k in range(KT)]
        whh1 = [wp.tile([128, 512], bf16, name=f"whh1_{k}") for k in range(KT)]
        xts = wp.tile([128, S], bf16, name="xts")
        gb0 = wp.tile([B, 512], bf16, name="gb0s")
        gb1 = wp.tile([B, 512], bf16, name="gb1s")
        idn = wp.tile([B, B], bf16, name="idns")
        encx = wp.tile([128, 512], bf16, name="encxs")
        h1store = wp.tile([128, KT * S], bf16, name="h1store")
        linw = [wp.tile([128, VS], bf16, name=f"linw_{k}") for k in range(KT + 1)]
        ones = wp.tile([1, 128], bf16, name="ones")

        for k in range(KT):
            nc.sync.dma_start(whh0[k][:], d_whh0[k])
            nc.sync.dma_start(wih1[k][:], d_wih1[k])
            nc.sync.dma_start(whh1[k][:], d_whh1[k])
        nc.sync.dma_start(a1[:], d_a1[:])
        nc.sync.dma_start(xts[:], d_xt[:])
        nc.sync.dma_start(gb0[:], d_gb0[:])
        nc.sync.dma_start(gb1[:], d_gb1[:])
        nc.sync.dma_start(idn[:], d_id[:])
        nc.sync.dma_start(encx[:], d_encx[:])
        cinit_sb = wp.tile([128, 32], f32, name="cinits")
        nc.sync.dma_start(cinit_sb[:], d_cinit[:])
        nc.gpsimd.memset(ones[:], 1.0)

        for _rep in range(reps):
            ct0 = cp.tile([128, 32], f32, tag="ct0")
            ct1 = cp.tile([128, 32], f32, tag="ct1")
            nc.sync.dma_start(ct0[:], d_cinit[:])
            nc.sync.dma_start(ct1[:], d_cinit[:])

            with tc.tile_pool(name="pg", bufs=3, space="PSUM") as pg:
                hx_m1 = encx        # [h0(tau-1) | h1(tau-2)] in [k, 64] blocks

                # Layer 1 lags layer 0 by one tick: tick tau runs L1 for
                # step tau-1 and L0 for step tau, then ONE AllGather ships
                # [h0_tau | h1_(tau-1)]; every matmul of a tick depends only
                # on the previous tick's AllGather.
                for tau in range(T + 1):
                    hs = tp.tile([128, 64], bf16, tag="hs")

                    # ---- layer 1 for t = tau-1 ----
                    if tau >= 1:
                        t1_ = tau - 1
                        g1 = pg.tile([128, 128], f32, tag="g1")
                        for m in range(MT):
                            msl = slice(m * 128, (m + 1) * 128)
                            nc.tensor.matmul(g1[:, m * 32:(m + 1) * 32],
                                             gb1[:, msl], idn[:],
                                             start=True, stop=False)
                            for k in range(2 * KT):
                                if k < KT:
                                    lhsT = whh1[k][:, msl]
                                    rhs = hx_m1[:, k * 64 + 32:k * 64 + 64]
                                else:
                                    lhsT = wih1[k - KT][:, msl]
                                    rhs = hx_m1[:, (k - KT) * 64:(k - KT) * 64 + 32]
                                nc.tensor.matmul(g1[:, m * 32:(m + 1) * 32],
                                                 lhsT, rhs, start=False,
                                                 stop=(k == 2 * KT - 1))
                        sg1 = tp.tile([128, 96], bf16, tag="sg1")
                        nc.scalar.activation(sg1[:], g1[:, 0:96], AF.Sigmoid)
                        tg1 = tp.tile([128, 32], bf16, tag="tg1")
                        nc.scalar.activation(tg1[:], g1[:, 96:128], AF.Tanh)
                        t2 = tp.tile([128, 32], f32, tag="t2")
                        nc.vector.tensor_mul(t2[:], sg1[:, 0:32], tg1[:])
                        ct1n = cp.tile([128, 32], f32, tag="ct1")
                        nc.vector.tensor_mul(ct1n[:], sg1[:, 32:64], ct1[:])
                        nc.vector.tensor_add(ct1n[:], ct1n[:], t2[:])
                        ct1 = ct1n
                        th1 = tp.tile([128, 32], bf16, tag="th1")
                        nc.scalar.activation(th1[:], ct1[:], AF.Tanh)
                        nc.vector.tensor_mul(hs[:, 32:64], sg1[:, 64:96], th1[:])
                    else:
                        nc.vector.tensor_copy(hs[:, 32:64], cinit_sb[:])

                    # ---- layer 0 for t = tau ----
                    if tau < T:
                        g0 = pg.tile([128, 128], f32, tag="g0")
                        for m in range(MT):
                            msl = slice(m * 128, (m + 1) * 128)
                            nc.tensor.matmul(g0[:, m * 32:(m + 1) * 32],
                                             gb0[:, msl], idn[:],
                                             start=True, stop=False)
                            for k in range(KT + 1):
                                lhsT = whh0[k][:, msl] if k < KT else a1[:, msl]
                                rhs = hx_m1[:, k * 64:k * 64 + 32] if k < KT \
                                    else xts[:, tau * 32:(tau + 1) * 32]
                                nc.tensor.matmul(g0[:, m * 32:(m + 1) * 32],
                                                 lhsT, rhs, start=False,
                                                 stop=(k == KT))
                        sg0 = tp.tile([128, 96], bf16, tag="sg0")
                        nc.scalar.activation(sg0[:], g0[:, 0:96], AF.Sigmoid)
                        tg0 = tp.tile([128, 32], bf16, tag="tg0")
                        nc.scalar.activation(tg0[:], g0[:, 96:128], AF.Tanh)
                        t1 = tp.tile([128, 32], f32, tag="t1")
                        nc.vector.tensor_mul(t1[:], sg0[:, 0:32], tg0[:])
                        ct0n = cp.tile([128, 32], f32, tag="ct0")
                        nc.vector.tensor_mul(ct0n[:], sg0[:, 32:64], ct0[:])
                        nc.vector.tensor_add(ct0n[:], ct0n[:], t1[:])
                        ct0 = ct0n
                        th0 = tp.tile([128, 32], bf16, tag="th0")
                        nc.scalar.activation(th0[:], ct0[:], AF.Tanh)
                        nc.vector.tensor_mul(hs[:, 0:32], sg0[:, 64:96], th0[:])
                    else:
                        nc.vector.tensor_copy(hs[:, 0:32], cinit_sb[:])

                    # ---- ONE AllGather for [h0_tau | h1_(tau-1)] ----
                    # h1 half ships while cell0 is still computing
                    bi = dp.tile([128, 64], bf16, tag="bi")
                    bo = dp.tile([NCORES, 128, 64], bf16, tag="bo",
                                 addr_space="Shared")
                    nc.sync.dma_start(bi[:, 32:64], hs[:, 32:64])
                    nc.sync.dma_start(bi[:, 0:32], hs[:, 0:32])
                    nc.gpsimd.collective_compute(
                        "AllGather", ALU.bypass, replica_groups=rg,
                        ins=[bi[:].opt()], outs=[bo[:].opt()])
                    if tau < T:
                        hx_new = hp.tile([128, 512], bf16, tag="hx")
                        nc.sync.dma_start(
                            hx_new[:].rearrange("p (k c) -> p k c", k=KT),
                            bo[:].rearrange("k p c -> p k c"))
                        hx_m1 = hx_new
                    if tau >= 1:
                        nc.sync.dma_start(
                            h1store[:].rearrange("p (k s) -> p k s", k=KT)
                            [:, :, (tau - 1) * 32:tau * 32],
                            bo[:].rearrange("k p c -> p k c")[:, :, 32:64])

            # ============== vocab projection + log_softmax ==============
            for k in range(KT + 1):
                nc.sync.dma_start(linw[k][:], d_linwt[k])

            with tc.tile_pool(name="vp", bufs=4, space="PSUM") as vp, \
                 tc.tile_pool(name="lg", bufs=5) as lgp, \
                 tc.tile_pool(name="ob", bufs=2) as obp, \
                 tc.tile_pool(name="ex", bufs=2) as exp_p, \
                 tc.tile_pool(name="tot", bufs=1) as totp:

                totals = totp.tile([128, 16], f32, name="totals")
                lse = totp.tile([128, 16], f32, name="lse")
                neglse = totp.tile([128, 16], f32, name="neglse")
                out_tb = d_out[:].rearrange("(b t) v -> t b v", b=B)

                lgt = {}
                for (c0, c1) in AR_CHUNKS:
                    for m in range(c0, c1):
                        M = 128 if m < NMT - 1 else S - 128 * (NMT - 1)
                        lg = lgp.tile([128, VS], bf16, tag="lg")
                        lgt[m] = (lg, M)
                        for c in range(NCHK):
                            ps = vp.tile([128, CHUNK], f32, tag="ps")
                            for k in range(KT + 1):
                                if k < KT:
                                    lhsT = h1store[:, k * S + m * 128:
                                                   k * S + m * 128 + M]
                                    rhs = linw[k][:, c * CHUNK:(c + 1) * CHUNK]
                                else:
                                    lhsT = ones[0:1, 0:M]
                                    rhs = linw[k][0:1, c * CHUNK:(c + 1) * CHUNK]
                                nc.tensor.matmul(ps[:M], lhsT, rhs,
                                                 start=(k == 0), stop=(k == KT))
                            nc.vector.tensor_copy(
                                lg[:M, c * CHUNK:(c + 1) * CHUNK], ps[:M])
                        ex = exp_p.tile([128, VS], bf16, tag="ex")
                        nc.scalar.activation(ex[:M], lg[:M, :], AF.Exp,
                                             accum_out=totals[:M, m:m + 1])

                    # one AllReduce for this chunk's per-sample exp-sums
                    nm = c1 - c0
                    ari = dp.tile([128, nm], f32, tag="ari")
                    aro = dp.tile([128, nm], f32, tag="aro", addr_space="Shared")
                    nc.sync.dma_start(ari[:], totals[:, c0:c1])
                    nc.gpsimd.collective_compute(
                        "AllReduce", ALU.add, replica_groups=rg,
                        ins=[ari[:].opt()], outs=[aro[:].opt()])
                    nc.sync.dma_start(lse[:, c0:c1], aro[:])
                    nc.scalar.activation(lse[:, c0:c1], lse[:, c0:c1], AF.Ln)
                    nc.vector.tensor_scalar_mul(neglse[:, c0:c1],
                                                lse[:, c0:c1], -1.0)

                    for m in range(c0, c1):
                        lg, M = lgt.pop(m)
                        q = M // 32
                        for h in range(2):
                            ob = obp.tile([128, VS // 2], f32, tag="ob")
                            if (m + h) % 2 == 0:
                                nc.vector.tensor_scalar(
                                    ob[:M],
                                    lg[:M, h * (VS // 2):(h + 1) * (VS // 2)],
                                    lse[:M, m:m + 1], None, op0=ALU.subtract)
                            else:
                                nc.scalar.activation(
                                    ob[:M],
                                    lg[:M, h * (VS // 2):(h + 1) * (VS // 2)],
                                    AF.Identity, bias=neglse[:M, m:m + 1])
                            dst = out_tb[m * 4:m * 4 + q, :,
                                         h * (VS // 2):(h + 1) * (VS // 2)]
                            nc.sync.dma_start(dst, ob[:M])

    nc.compile()
    return nc


def _get_nc(reps=1):
    key = ("nc", reps)
    if key not in _BUILD_CACHE:
        _BUILD_CACHE[key] = _build(reps)
    return _BUILD_CACHE[key]


def run(inputs, trace=False):
    from concourse.bass_utils import run_bass_kernel_spmd

    in_maps = _host_prep(inputs)
    nc = _get_nc()
    res = run_bass_kernel_spmd(nc, in_maps, core_ids=list(range(NCORES)),
                               trace=trace)
    full = np.empty((S, V), np.float32)
    for r in range(NCORES):
        full[:, r * VS:(r + 1) * VS] = res.results[r]["out"]
    return full, res


def kernel(**inputs):
    full, _ = run(inputs)
    return full

